# revision 22
# baseline (speedup 1.0000x reference)
"""Multi-head causal attention (B=4, T=2048, D=1024, H=16, hd=64) on 8 trn2 cores.

Sharding: core = (batch, head_group): 4 batches x 2 head-groups of 8 heads.
Each core computes its batch's attention for its 8 heads plus the partial
output projection; the host sums the two head-group partials per batch and
adds the output bias.

Per-core kernel (all activations kept transposed, [feature, token]):
  QT = Wq_s^T @ x^T        [512, 2048]   (PE, fp32r)
  KT = Wk_s^T @ x^T        [512, 2048]
  V  = x @ Wv_s            [2048, 512]   (token-partition layout, +ones col)
  per head h, i-chunk c (1024 wide), j-tile t (128 rows):
    S^T[j, i] = KT_h[:, jtile]^T-contract QT_h    (PE, causal extents only)
    expS = exp(S^T / 8)                           (ACT, PSUM->SBUF)
    causal mask on the 128-col diagonal block     (GPSIMD affine_select)
    ctxT_aug += V_aug[jtile]^T-contract expS      (PE, PSUM accumulate;
                                                   row 64 = softmax denom)
    ctx = ctxT[0:64] * (1/denom)                  (DVE + GPSIMD broadcast)
  out_partial = ctx^T-contract Wo_s               [2048, 1024]
"""

import os
import sys

sys.path.insert(0, "/opt/trn_rl_repo")

import numpy as np

B = 4
T = 2048
D = 1024
H = 16
HD = 64
NCORES = 8
HPC = 8          # heads per core
DPC = HPC * HD   # 512
KT = D // 128    # 8 k-tiles
NT = T // 128    # 16 token tiles

_CACHE = {}
LAST_RESULTS = None


def _build_program():
    from contextlib import ExitStack

    import concourse.bass as bass
    import concourse.tile as tile
    from concourse import bacc, mybir

    f32 = mybir.dt.float32
    f32r = mybir.dt.float32r
    bf16 = mybir.dt.bfloat16
    Exp = mybir.ActivationFunctionType.Exp

    def r(ap):
        return ap

    nc = bacc.Bacc(
        "TRN2", target_bir_lowering=False, debug=False, num_devices=NCORES
    )
    xT = nc.dram_tensor("xT", [D, T], f32r, kind="ExternalInput").ap()
    wq_d = nc.dram_tensor("wq", [D, DPC], f32r, kind="ExternalInput").ap()
    wk_d = nc.dram_tensor("wk", [D, DPC], f32r, kind="ExternalInput").ap()
    wv_d = nc.dram_tensor("wv", [D, DPC], f32r, kind="ExternalInput").ap()
    wo_d = nc.dram_tensor("wo", [DPC, D], f32r, kind="ExternalInput").ap()
    out_d = nc.dram_tensor("out", [T, D], f32, kind="ExternalOutput").ap()

    with tile.TileContext(nc) as tc, ExitStack() as top:
        persist = top.enter_context(tc.tile_pool(name="persist", bufs=1))
        qt = persist.tile([128, 4, T], f32r, tag="qt")
        kt = persist.tile([128, 4, T], f32r, tag="kt")
        v_sb = persist.tile([128, NT, HPC * (HD + 1)], f32r, tag="v")

        # ones columns for the softmax-denominator rows of the PV matmul
        # (memset can't emit f32r; go through an f32 scratch + rounding copy)
        ones_f32 = persist.tile([128, NT, 1], f32, tag="ones")
        nc.vector.memset(ones_f32, 1.0)
        for h in range(HPC):
            nc.vector.tensor_copy(
                v_sb[:, :, 65 * h + 64 : 65 * h + 65], ones_f32
            )

        # ---------------- phase 1: QT / KT / V projections ----------------
        with ExitStack() as ph1:
            wpool = ph1.enter_context(tc.tile_pool(name="wpool", bufs=1))
            xpool = ph1.enter_context(tc.tile_pool(name="xpool", bufs=2))
            ps1 = ph1.enter_context(tc.tile_pool(name="ps1", bufs=4, space="PSUM"))

            wq = wpool.tile([128, KT, DPC], f32r, tag="wq")
            wk = wpool.tile([128, KT, DPC], f32r, tag="wk")
            wv = wpool.tile([128, KT, DPC], f32r, tag="wv")
            # split per k-tile so the first matmuls start early; use the
            # gpsimd DMA queue so weight loads overlap the x-chunk loads
            # that flow through the sync queue
            for k in range(KT):
                for w_sb, w_d in ((wv, wv_d), (wq, wq_d), (wk, wk_d)):
                    nc.gpsimd.dma_start(
                        out=w_sb[:, k, :],
                        in_=w_d.rearrange("(k p) d -> p k d", p=128)[:, k, :],
                    )

            xT_r = xT.rearrange("(k p) t -> p k t", p=128)
            XC = 256  # token-chunk width for the projection phase
            for tci in range(T // XC):
                xt = xpool.tile([128, KT, XC], f32r, tag="xt")
                nc.sync.dma_start(
                    out=xt, in_=xT_r[:, :, XC * tci : XC * (tci + 1)]
                )
                for tt4 in range(XC // 128):
                    tt = (XC // 128) * tci + tt4
                    ps = ps1.tile([128, DPC], f32, tag="psv")
                    for k in range(KT):
                        nc.tensor.matmul(
                            ps,
                            r(xt[:, k, 128 * tt4 : 128 * (tt4 + 1)]),
                            r(wv[:, k, :]),
                            start=(k == 0),
                            stop=(k == KT - 1),
                        )
                    nc.vector.tensor_copy(
                        v_sb[:, tt, :].rearrange("p (h c) -> p h c", c=HD + 1)[
                            :, :, 0:HD
                        ],
                        ps.rearrange("p (h c) -> p h c", c=HD),
                    )
                for w_sb, dest in ((wq, qt), (wk, kt)):
                    for dt in range(4):
                        ps = ps1.tile([128, XC], f32, tag="ps1")
                        for k in range(KT):
                            nc.tensor.matmul(
                                ps,
                                r(w_sb[:, k, 128 * dt : 128 * (dt + 1)]),
                                r(xt[:, k, :]),
                                start=(k == 0),
                                stop=(k == KT - 1),
                            )
                        nc.vector.tensor_copy(
                            dest[:, dt, XC * tci : XC * (tci + 1)], ps
                        )

        # ---------------- phase 2: attention ----------------
        # prefetch Wo while attention runs
        wop = top.enter_context(tc.tile_pool(name="wop", bufs=1))
        wo = wop.tile([128, 4, D], f32r, tag="wo")
        nc.sync.dma_start(out=wo, in_=wo_d.rearrange("(c p) o -> p c o", p=128))

        ctx_sb = persist.tile([128, 4, T], f32r, tag="ctx")

        with ExitStack() as ph2:
            expp = ph2.enter_context(tc.tile_pool(name="expp", bufs=3))
            smallp = ph2.enter_context(tc.tile_pool(name="smallp", bufs=2))
            pss = ph2.enter_context(tc.tile_pool(name="pss", bufs=1, space="PSUM"))
            psc = ph2.enter_context(tc.tile_pool(name="psc", bufs=2, space="PSUM"))
            pso = ph2.enter_context(tc.tile_pool(name="pso", bufs=2, space="PSUM"))
            outp = ph2.enter_context(tc.tile_pool(name="outp", bufs=4))

            def normalize(ctx_ps, h, c):
                """Move ctx out of PSUM and divide rows 0..63 by row 64,
                without any long-latency op on the inter-head critical path.
                The reciprocal of the [1, 1024] sums row would take ~8 cycles
                per element on a single DVE lane; spread it over 128 lanes
                with a pair of tiny SBUF->SBUF DMA reshapes instead."""
                dq = h // 2
                pr = (h % 2) * 64
                raw = smallp.tile([65, 1024], f32, tag="raw")
                nc.vector.tensor_copy(raw, ctx_ps)
                sp = smallp.tile([128, 8], f32, tag="sp")
                nc.sync.dma_start(out=sp, in_=raw[64:65, :])
                rp = smallp.tile([128, 8], f32, tag="rp")
                nc.vector.reciprocal(rp, sp)
                recip = smallp.tile([1, 1024], f32, tag="recip")
                nc.sync.dma_start(out=recip, in_=rp)
                bc = smallp.tile([64, 1024], f32, tag="bc")
                nc.gpsimd.partition_broadcast(bc, recip)
                nc.vector.tensor_mul(
                    ctx_sb[pr : pr + 64, dq, 1024 * c : 1024 * (c + 1)],
                    raw[0:64, :],
                    bc,
                )

            for c in range(2):
                for m in range(4):  # head pair (2m, 2m+1) shares qt/kt tile m
                    ctx_pair = [
                        psc.tile([65, 1024], f32, tag="ctxps", name=f"ctxps_{m}_{c}_{i}")
                        for i in range(2)
                    ]
                    for t in range(8 * c + 8):
                        i0 = max(128 * t, 1024 * c)
                        ext = 1024 * (c + 1) - i0
                        for half in range(2):
                            h = 2 * m + half
                            pr = half * 64
                            s_ps = pss.tile([128, 1024], f32, tag="sps")
                            col = 0
                            while col < ext:
                                w = min(512, ext - col)
                                nc.tensor.matmul(
                                    s_ps[:, col : col + w],
                                    kt[pr : pr + 64, m, 128 * t : 128 * (t + 1)],
                                    qt[pr : pr + 64, m, i0 + col : i0 + col + w],
                                    start=True,
                                    stop=True,
                                )
                                col += w
                            es = expp.tile([128, 1024], f32r, tag="es")
                            nc.scalar.activation(
                                es[:, 0:ext], s_ps[:, 0:ext], Exp, scale=0.125
                            )
                            if i0 == 128 * t:
                                # keep element iff free_idx - partition_idx >= 0
                                nc.gpsimd.affine_select(
                                    out=es[:, 0:128],
                                    in_=es[:, 0:128],
                                    compare_op=mybir.AluOpType.is_ge,
                                    fill=0.0,
                                    base=0,
                                    pattern=[[1, 128]],
                                    channel_multiplier=-1,
                                )
                            for s in range(2):
                                cs = 1024 * c + 512 * s
                                lo = max(i0, cs)
                                hi = cs + 512
                                if lo >= hi:
                                    continue
                                nc.tensor.matmul(
                                    ctx_pair[half][:, lo - 1024 * c : hi - 1024 * c],
                                    v_sb[:, t, 65 * h : 65 * h + 65],
                                    es[:, lo - i0 : hi - i0],
                                    start=(t == 0),
                                    stop=(t == hi // 128 - 1),
                                )
                    for half in range(2):
                        normalize(ctx_pair[half], 2 * m + half, c)

                # output projection for token chunk c: emitted here so the
                # scheduler can use these dense matmuls to fill the PE's
                # exp-dependency gaps during the next chunk's attention
                for tt in range(8 * c, 8 * c + 8):
                    for oc in range(2):
                        ps = pso.tile([128, 512], f32, tag="pso")
                        for ct in range(4):
                            nc.tensor.matmul(
                                ps,
                                r(ctx_sb[:, ct, 128 * tt : 128 * (tt + 1)]),
                                r(wo[:, ct, 512 * oc : 512 * (oc + 1)]),
                                start=(ct == 0),
                                stop=(ct == 3),
                            )
                        ot = outp.tile([128, 512], f32, tag="ot")
                        nc.vector.tensor_copy(ot, ps)
                        nc.sync.dma_start(
                            out=out_d[
                                128 * tt : 128 * (tt + 1), 512 * oc : 512 * (oc + 1)
                            ],
                            in_=ot,
                        )

    nc.compile()
    return nc


def _get_program():
    if "nc" not in _CACHE:
        _CACHE["nc"] = _build_program()
    return _CACHE["nc"]


def make_in_maps(x, Wq, Wk, Wv, Wo):
    in_maps = []
    for core in range(NCORES):
        b, hg = core // 2, core % 2
        sl = slice(DPC * hg, DPC * (hg + 1))
        in_maps.append(
            {
                "xT": np.ascontiguousarray(x[b].T),
                "wq": np.ascontiguousarray(Wq[:, sl]),
                "wk": np.ascontiguousarray(Wk[:, sl]),
                "wv": np.ascontiguousarray(Wv[:, sl]),
                "wo": np.ascontiguousarray(Wo[sl, :]),
            }
        )
    return in_maps


def kernel(x, Wq, Wk, Wv, Wo, bo):
    global LAST_RESULTS
    from concourse.bass_utils import run_bass_kernel_spmd

    x = np.asarray(x, dtype=np.float32)
    nc = _get_program()
    in_maps = make_in_maps(
        x,
        np.asarray(Wq, np.float32),
        np.asarray(Wk, np.float32),
        np.asarray(Wv, np.float32),
        np.asarray(Wo, np.float32),
    )
    res = run_bass_kernel_spmd(
        nc,
        in_maps,
        list(range(NCORES)),
        trace=bool(int(os.environ.get("KERNEL_TRACE", "0"))),
    )
    LAST_RESULTS = res
    bo = np.asarray(bo, np.float32)
    out = np.empty((B, T, D), np.float32)
    for b in range(B):
        out[b] = res.results[2 * b]["out"] + res.results[2 * b + 1]["out"] + bo
    return out


# revision 23
# speedup vs baseline: 1.5127x; 1.5127x over previous
"""Multi-head causal attention (B=4, T=2048, D=1024, H=16, hd=64) on 8 trn2 cores.

Sharding: core = (batch, head_group): 4 batches x 2 head-groups of 8 heads.
Each core computes its batch's attention for its 8 heads plus the partial
output projection; the host sums the two head-group partials per batch and
adds the output bias.

Per-core kernel (all activations kept transposed, [feature, token]):
  QT = Wq_s^T @ x^T        [512, 2048]   (PE, fp32r)
  KT = Wk_s^T @ x^T        [512, 2048]
  V  = x @ Wv_s            [2048, 512]   (token-partition layout, +ones col)
  per head h, i-chunk c (1024 wide), j-tile t (128 rows):
    S^T[j, i] = KT_h[:, jtile]^T-contract QT_h    (PE, causal extents only)
    expS = exp(S^T / 8)                           (ACT, PSUM->SBUF)
    causal mask on the 128-col diagonal block     (GPSIMD affine_select)
    ctxT_aug += V_aug[jtile]^T-contract expS      (PE, PSUM accumulate;
                                                   row 64 = softmax denom)
    ctx = ctxT[0:64] * (1/denom)                  (DVE + GPSIMD broadcast)
  out_partial = ctx^T-contract Wo_s               [2048, 1024]
"""

import os
import sys

sys.path.insert(0, "/opt/trn_rl_repo")

import numpy as np

B = 4
T = 2048
D = 1024
H = 16
HD = 64
NCORES = 8
HPC = 8          # heads per core
DPC = HPC * HD   # 512
KT = D // 128    # 8 k-tiles
NT = T // 128    # 16 token tiles

_CACHE = {}
LAST_RESULTS = None


def _build_program():
    from contextlib import ExitStack

    import concourse.bass as bass
    import concourse.tile as tile
    from concourse import bacc, mybir

    f32 = mybir.dt.float32
    f32r = mybir.dt.float32r
    bf16 = mybir.dt.bfloat16
    Exp = mybir.ActivationFunctionType.Exp

    def r(ap):
        return ap

    nc = bacc.Bacc(
        "TRN2", target_bir_lowering=False, debug=False, num_devices=NCORES
    )
    xT = nc.dram_tensor("xT", [D, T], f32r, kind="ExternalInput").ap()
    wq_d = nc.dram_tensor("wq", [D, DPC], f32r, kind="ExternalInput").ap()
    wk_d = nc.dram_tensor("wk", [D, DPC], f32r, kind="ExternalInput").ap()
    wv_d = nc.dram_tensor("wv", [D, DPC], f32r, kind="ExternalInput").ap()
    wo_d = nc.dram_tensor("wo", [DPC, D], f32r, kind="ExternalInput").ap()
    out_d = nc.dram_tensor("out", [T, D], f32, kind="ExternalOutput").ap()

    with tile.TileContext(nc) as tc, ExitStack() as top:
        persist = top.enter_context(tc.tile_pool(name="persist", bufs=1))
        qt = persist.tile([128, 4, T], f32r, tag="qt")
        kt = persist.tile([128, 4, T], f32r, tag="kt")
        v_sb = persist.tile([128, NT, HPC * (HD + 1)], f32r, tag="v")

        # ones columns for the softmax-denominator rows of the PV matmul
        # (memset can't emit f32r; go through an f32 scratch + rounding copy)
        ones_f32 = persist.tile([128, NT, 1], f32, tag="ones")
        nc.vector.memset(ones_f32, 1.0)
        for h in range(HPC):
            nc.vector.tensor_copy(
                v_sb[:, :, 65 * h + 64 : 65 * h + 65], ones_f32
            )

        # ---------------- phase 1: QT / KT / V projections ----------------
        with ExitStack() as ph1:
            wpool = ph1.enter_context(tc.tile_pool(name="wpool", bufs=1))
            xpool = ph1.enter_context(tc.tile_pool(name="xpool", bufs=2))
            ps1 = ph1.enter_context(tc.tile_pool(name="ps1", bufs=4, space="PSUM"))

            wq = wpool.tile([128, KT, DPC], f32r, tag="wq")
            wk = wpool.tile([128, KT, DPC], f32r, tag="wk")
            wv = wpool.tile([128, KT, DPC], f32r, tag="wv")
            # split per k-tile so the first matmuls start early; use the
            # gpsimd DMA queue so weight loads overlap the x-chunk loads
            # that flow through the sync queue
            for k in range(KT):
                for w_sb, w_d in ((wv, wv_d), (wq, wq_d), (wk, wk_d)):
                    nc.gpsimd.dma_start(
                        out=w_sb[:, k, :],
                        in_=w_d.rearrange("(k p) d -> p k d", p=128)[:, k, :],
                    )

            xT_r = xT.rearrange("(k p) t -> p k t", p=128)
            XC = 256  # token-chunk width for the projection phase
            for tci in range(T // XC):
                xt = xpool.tile([128, KT, XC], f32r, tag="xt")
                nc.sync.dma_start(
                    out=xt, in_=xT_r[:, :, XC * tci : XC * (tci + 1)]
                )
                for tt4 in range(XC // 128):
                    tt = (XC // 128) * tci + tt4
                    ps = ps1.tile([128, DPC], f32, tag="psv")
                    for k in range(KT):
                        nc.tensor.matmul(
                            ps,
                            r(xt[:, k, 128 * tt4 : 128 * (tt4 + 1)]),
                            r(wv[:, k, :]),
                            start=(k == 0),
                            stop=(k == KT - 1),
                        )
                    nc.vector.tensor_copy(
                        v_sb[:, tt, :].rearrange("p (h c) -> p h c", c=HD + 1)[
                            :, :, 0:HD
                        ],
                        ps.rearrange("p (h c) -> p h c", c=HD),
                    )
                for w_sb, dest in ((wq, qt), (wk, kt)):
                    for dt in range(4):
                        ps = ps1.tile([128, XC], f32, tag="ps1")
                        for k in range(KT):
                            nc.tensor.matmul(
                                ps,
                                r(w_sb[:, k, 128 * dt : 128 * (dt + 1)]),
                                r(xt[:, k, :]),
                                start=(k == 0),
                                stop=(k == KT - 1),
                            )
                        nc.vector.tensor_copy(
                            dest[:, dt, XC * tci : XC * (tci + 1)], ps
                        )

        # ---------------- phase 2: attention ----------------
        # prefetch Wo while attention runs
        wop = top.enter_context(tc.tile_pool(name="wop", bufs=1))
        wo = wop.tile([128, 4, D], f32r, tag="wo")
        nc.sync.dma_start(out=wo, in_=wo_d.rearrange("(c p) o -> p c o", p=128))

        ctx_sb = persist.tile([128, 4, T], f32r, tag="ctx")

        with ExitStack() as ph2:
            expp = ph2.enter_context(tc.tile_pool(name="expp", bufs=3))
            smallp = ph2.enter_context(tc.tile_pool(name="smallp", bufs=2))
            pss = ph2.enter_context(tc.tile_pool(name="pss", bufs=2, space="PSUM"))
            psc = ph2.enter_context(tc.tile_pool(name="psc", bufs=2, space="PSUM"))

            def normalize(ctx_ps, h, c):
                """Move ctx out of PSUM and divide rows 0..63 by row 64,
                without any long-latency op on the inter-head critical path.
                The reciprocal of the [1, 1024] sums row would take ~8 cycles
                per element on a single DVE lane; spread it over 128 lanes
                with a pair of tiny SBUF->SBUF DMA reshapes instead."""
                dq = h // 2
                pr = (h % 2) * 64
                raw = smallp.tile([65, 1024], f32, tag="raw")
                nc.vector.tensor_copy(raw, ctx_ps)
                sp = smallp.tile([128, 8], f32, tag="sp")
                nc.sync.dma_start(out=sp, in_=raw[64:65, :])
                rp = smallp.tile([128, 8], f32, tag="rp")
                nc.vector.reciprocal(rp, sp)
                recip = smallp.tile([1, 1024], f32, tag="recip")
                nc.sync.dma_start(out=recip, in_=rp)
                bc = smallp.tile([64, 1024], f32, tag="bc")
                nc.gpsimd.partition_broadcast(bc, recip)
                nc.vector.tensor_mul(
                    ctx_sb[pr : pr + 64, dq, 1024 * c : 1024 * (c + 1)],
                    raw[0:64, :],
                    bc,
                )

            for m in range(4):  # head pair (2m, 2m+1) shares qt/kt tile m
                for c in range(2):
                    ctx_pair = [
                        psc.tile([65, 1024], f32, tag="ctxps", name=f"ctxps_{m}_{c}_{i}")
                        for i in range(2)
                    ]
                    for t in range(8 * c + 8):
                        i0 = max(128 * t, 1024 * c)
                        ext = 1024 * (c + 1) - i0
                        for half in range(2):
                            h = 2 * m + half
                            pr = half * 64
                            s_ps = pss.tile([128, 1024], f32, tag="sps")
                            col = 0
                            while col < ext:
                                w = min(512, ext - col)
                                nc.tensor.matmul(
                                    s_ps[:, col : col + w],
                                    kt[pr : pr + 64, m, 128 * t : 128 * (t + 1)],
                                    qt[pr : pr + 64, m, i0 + col : i0 + col + w],
                                    start=True,
                                    stop=True,
                                )
                                col += w
                            es = expp.tile([128, 1024], f32r, tag="es")
                            nc.scalar.activation(
                                es[:, 0:ext], s_ps[:, 0:ext], Exp, scale=0.125
                            )
                            if i0 == 128 * t:
                                # keep element iff free_idx - partition_idx >= 0
                                nc.gpsimd.affine_select(
                                    out=es[:, 0:128],
                                    in_=es[:, 0:128],
                                    compare_op=mybir.AluOpType.is_ge,
                                    fill=0.0,
                                    base=0,
                                    pattern=[[1, 128]],
                                    channel_multiplier=-1,
                                )
                            for s in range(2):
                                cs = 1024 * c + 512 * s
                                lo = max(i0, cs)
                                hi = cs + 512
                                if lo >= hi:
                                    continue
                                nc.tensor.matmul(
                                    ctx_pair[half][:, lo - 1024 * c : hi - 1024 * c],
                                    v_sb[:, t, 65 * h : 65 * h + 65],
                                    es[:, lo - i0 : hi - i0],
                                    start=(t == 0),
                                    stop=(t == hi // 128 - 1),
                                )
                    for half in range(2):
                        normalize(ctx_pair[half], 2 * m + half, c)

        # ---------------- phase 3: output projection ----------------
        with ExitStack() as ph3:
            outp = ph3.enter_context(tc.tile_pool(name="outp", bufs=6))
            pso = ph3.enter_context(tc.tile_pool(name="pso", bufs=4, space="PSUM"))
            for tt in range(NT):
                for oc in range(2):
                    ps = pso.tile([128, 512], f32, tag="pso")
                    for ct in range(4):
                        nc.tensor.matmul(
                            ps,
                            r(ctx_sb[:, ct, 128 * tt : 128 * (tt + 1)]),
                            r(wo[:, ct, 512 * oc : 512 * (oc + 1)]),
                            start=(ct == 0),
                            stop=(ct == 3),
                        )
                    ot = outp.tile([128, 512], f32, tag="ot")
                    nc.vector.tensor_copy(ot, ps)
                    nc.sync.dma_start(
                        out=out_d[
                            128 * tt : 128 * (tt + 1), 512 * oc : 512 * (oc + 1)
                        ],
                        in_=ot,
                    )

    nc.compile()
    return nc


def _get_program():
    if "nc" not in _CACHE:
        _CACHE["nc"] = _build_program()
    return _CACHE["nc"]


def make_in_maps(x, Wq, Wk, Wv, Wo):
    in_maps = []
    for core in range(NCORES):
        b, hg = core // 2, core % 2
        sl = slice(DPC * hg, DPC * (hg + 1))
        in_maps.append(
            {
                "xT": np.ascontiguousarray(x[b].T),
                "wq": np.ascontiguousarray(Wq[:, sl]),
                "wk": np.ascontiguousarray(Wk[:, sl]),
                "wv": np.ascontiguousarray(Wv[:, sl]),
                "wo": np.ascontiguousarray(Wo[sl, :]),
            }
        )
    return in_maps


def kernel(x, Wq, Wk, Wv, Wo, bo):
    global LAST_RESULTS
    from concourse.bass_utils import run_bass_kernel_spmd

    x = np.asarray(x, dtype=np.float32)
    nc = _get_program()
    in_maps = make_in_maps(
        x,
        np.asarray(Wq, np.float32),
        np.asarray(Wk, np.float32),
        np.asarray(Wv, np.float32),
        np.asarray(Wo, np.float32),
    )
    res = run_bass_kernel_spmd(
        nc,
        in_maps,
        list(range(NCORES)),
        trace=bool(int(os.environ.get("KERNEL_TRACE", "0"))),
    )
    LAST_RESULTS = res
    bo = np.asarray(bo, np.float32)
    out = np.empty((B, T, D), np.float32)
    for b in range(B):
        out[b] = res.results[2 * b]["out"] + res.results[2 * b + 1]["out"] + bo
    return out
